# revision 12
# baseline (speedup 1.0000x reference)
"""Trainium2 Bass kernel for masked two-template sparse attention.

Model (per sample, fp32 reference):
    qkv = (x @ W_qkv.T) * mask          mask: temp_mask on first 64 tokens, 1 elsewhere
    q,k,v split into 12 heads x 64
    template tokens (first 128) attend to template tokens only
    search tokens (last 324) attend to all 452 tokens
    out = concat(attn outputs) @ W_proj.T + b_proj

Sharding: data-parallel over batch, 32 samples -> 4 per NeuronCore x 8 cores.
All attention math in "transposed" layout (channels on partitions):
    x^T (PE transpose) -> q^T,k^T = Wqkv^T.T @ x^T ; v natural = x^T.T @ Wv^T
    S^T = k^T.T @ q^T   (head pairs row-tiled at (0,0)/(64,0), bf16 PSUM out)
    E^T = exp(S^T * scale)              (no max subtraction; |S| <~ 6)
    PV col-tiled: head pair h0/h1 -> partitions 0:64 / 64:128 of ONE tile,
      attn.V into bank0, ones.V (softmax denominators) into bank1
    attn^T = attn^T_unnorm * recip(sums)   (full 128-partition DVE ops)
    y = attn^T.T @ Wp^T                 (+ bias added on host)

Engine split: PE matmuls; ACT exp + PSUM->SBUF copies of qk/y; DVE x^T/v
copies + normalize; Pool (GpSimd) mask multiply + fp32->bf16 cast.
"""

import numpy as np
import ml_dtypes

import concourse.bass as bass
import concourse.mybir as mybir
import concourse.tile as tile
from concourse.bass_utils import run_bass_kernel_spmd
from concourse.masks import make_identity

# ---------------- configuration ----------------
PROJ_DT_NAME = "bfloat16"
ATT_DT_NAME = "bfloat16"
TRACE = False        # request NTFF profile on run
PHASES = 99          # kept for test.py compat (unused)
REPS = 1             # timing: repeat the whole computation inside the NEFF

NCORES = 8
S = 4                # samples per core
N, C, H, DH = 452, 768, 12, 64
NMT, NS = 128, 324   # template tokens / search tokens
SCALE = DH ** -0.5
TCH = [(0, 128), (128, 256), (256, 384), (384, 452)]  # token chunks
KC = 6               # channel chunks of 128
NPAD = 512           # padded token width for x^T storage

_F32 = mybir.dt.float32


def _legalize_waits(nc, max_waits=1):
    """This container's walrus accepts at most one sync-wait per instruction;
    hoist extra waits onto dedicated NOPs in front of the instruction."""
    n_split = 0
    for f in nc.m.functions:
        for bb in f.blocks:
            new_insts = []
            for inst in bb.instructions:
                si = inst.sync_info
                if si is not None and si.on_wait and len(si.on_wait) > max_waits:
                    waits = list(si.on_wait)
                    for i, w in enumerate(waits[:-max_waits]):
                        new_insts.append(
                            mybir.InstNoOp(
                                name=f"{inst.name}-w{i}",
                                sync_info=mybir.SyncInfo(on_wait=[w], on_update=[]),
                                bass_nofuse=True,
                                engine=inst.engine,
                            )
                        )
                    si.on_wait = waits[-max_waits:]
                    n_split += 1
                new_insts.append(inst)
            bb.instructions = new_insts
    return n_split


def build_module():
    pdt = getattr(mybir.dt, PROJ_DT_NAME)
    adt = getattr(mybir.dt, ATT_DT_NAME)

    nc = bass.Bass("TRN2", target_bir_lowering=False, debug=False)
    x_d = nc.dram_tensor("x", [S, N, C], _F32, kind="ExternalInput").ap()
    m_d = nc.dram_tensor("tmask", [S, 64], _F32, kind="ExternalInput").ap()
    wq_d = nc.dram_tensor("wqkvT", [C, 3 * C], pdt, kind="ExternalInput").ap()
    wp_d = nc.dram_tensor("wpT", [C, C], pdt, kind="ExternalInput").ap()
    y_d = nc.dram_tensor("y", [S, N, C], _F32, kind="ExternalOutput").ap()

    Exp = mybir.ActivationFunctionType.Exp

    with tile.TileContext(nc) as tc:
        with (
            tc.tile_pool(name="const", bufs=1) as cp,
            tc.tile_pool(name="work", bufs=1) as wk,
            # PSUM: 8 banks total.
            #  ptr: 1-bank bf16 tiles - x^T transposes.    2 bufs -> 2 banks
            #  pacc: 1-bank f32 tiles - qk/v/y projections. 2 bufs -> 2 banks
            #  ppv: 2-bank f32 tiles - scores AND PV out share the tag
            #       (5 allocs per head pair).              2 bufs -> 4 banks
            tc.tile_pool(name="ptr", bufs=2, space="PSUM") as ptp,
            tc.tile_pool(name="pacc", bufs=2, space="PSUM") as pacc,
            tc.tile_pool(name="ppv", bufs=2, space="PSUM") as ppv,
        ):
            # ---- persistent constants ----
            wq_sb = []
            for i in range(KC):
                w = cp.tile([128, 3 * C], pdt, name=f"wq{i}", tag=f"wq{i}")
                nc.scalar.dma_start(w[:, :], wq_d[i * 128:(i + 1) * 128, :])
                wq_sb.append(w)
            wp_sb = []
            for i in range(KC):
                w = cp.tile([128, C], pdt, name=f"wp{i}", tag=f"wp{i}")
                nc.scalar.dma_start(w[:, :], wp_d[i * 128:(i + 1) * 128, :])
                wp_sb.append(w)
            ident = cp.tile([128, 128], adt, name="ident", tag="ident")
            make_identity(nc, ident)
            ones = cp.tile([128, 64], adt, name="ones", tag="ones")
            nc.gpsimd.memset(ones[:, :], 1.0)

            for s in [si for _rep in range(REPS) for si in range(S)]:
                # ---- load + mask + cast (Pool) ----
                xn = wk.tile([128, 4, C], _F32, name="xn", tag="xn", bufs=2)
                # zero the token-pad rows first; the tail DMA then overwrites
                # rows 64:68 with real data (partition bases must be 32-aligned)
                nc.gpsimd.memset(xn[64:128, 3, :], 0.0)
                nc.sync.dma_start(
                    xn[:, 0:3, :],
                    x_d[s, 0:384, :].rearrange("(c p) d -> p c d", p=128),
                )
                nc.sync.dma_start(xn[0:68, 3, :], x_d[s, 384:452, :])
                msk = wk.tile([64, 1], _F32, name="msk", tag="msk", bufs=2)
                nc.sync.dma_start(msk[:, :], m_d[s, :].unsqueeze(1))
                nc.gpsimd.tensor_scalar_mul(xn[0:64, 0, :], xn[0:64, 0, :], msk[0:64, :])
                xnc = wk.tile([128, 4, C], adt, name="xnc", tag="xnc", bufs=2)
                nc.gpsimd.tensor_copy(xnc[:, :, :], xn[:, :, :])

                # ---- x^T via PE transpose (full 128 rows incl zero pad) ----
                xTb = wk.tile([128, KC, NPAD], pdt, name="xTb", tag="xTb", bufs=2)
                for ti in range(4):
                    ptr = ptp.tile([128, 1024], adt, name="ptr", tag="tr")
                    for cc in range(KC):
                        nc.tensor.transpose(
                            ptr[:, cc * 128:(cc + 1) * 128],
                            xnc[:, ti, cc * 128:(cc + 1) * 128],
                            ident[:, :],
                        )
                    nc.vector.tensor_copy(
                        xTb[:, :, ti * 128:(ti + 1) * 128],
                        ptr.rearrange("p (c k) -> p c k", k=128)[:, 0:KC, :],
                    )
                xT = [xTb[:, cc, 0:N] for cc in range(KC)]

                # ---- q^T / k^T projections (12 chunks of 128 channels) ----
                qkT = []
                for oc in range(12):
                    pq = pacc.tile([128, 512], _F32, name="pq", tag="acc")
                    for kc in range(KC):
                        nc.tensor.matmul(
                            pq[:, 0:N],
                            wq_sb[kc][:, oc * 128:(oc + 1) * 128],
                            xT[kc],
                            start=(kc == 0),
                            stop=(kc == KC - 1),
                        )
                    t = wk.tile([128, N], adt, name=f"qkT{oc}", tag=f"qkT{oc}", bufs=2)
                    nc.vector.tensor_copy(t[:, :], pq[:, 0:N])
                    qkT.append(t)

                # ---- v projection (token-major), per head 64 contiguous cols ----
                vt = []
                for ti, (t0, t1) in enumerate(TCH):
                    tsz = t1 - t0
                    pva = pacc.tile([128, 512], _F32, name="pva", tag="acc")
                    pvb = pacc.tile([128, 512], _F32, name="pvb", tag="acc")
                    for kc in range(KC):
                        nc.tensor.matmul(
                            pva[0:tsz, 0:512],
                            xTb[:, kc, t0:t1],
                            wq_sb[kc][:, 1536:2048],
                            start=(kc == 0),
                            stop=(kc == KC - 1),
                        )
                        nc.tensor.matmul(
                            pvb[0:tsz, 0:256],
                            xTb[:, kc, t0:t1],
                            wq_sb[kc][:, 2048:2304],
                            start=(kc == 0),
                            stop=(kc == KC - 1),
                        )
                    t = wk.tile([128, C], adt, name=f"v{ti}", tag=f"v{ti}", bufs=2)
                    nc.vector.tensor_copy(t[0:tsz, 0:512], pva[0:tsz, 0:512])
                    nc.vector.tensor_copy(t[0:tsz, 512:768], pvb[0:tsz, 0:256])
                    vt.append(t)

                # ---- attention, head pairs ----
                attnT = []
                for p in range(6):
                    qc, kt = qkT[p], qkT[6 + p]
                    # scores^T, f32 PSUM. Per (head, half): 2-bank tile with
                    # search j-chunks at 0:324 / 512:836, template at 836:964
                    # (half 0 only). Head pair MMs are row-tiled at (0,0) /
                    # (64,0) into different buffers and run concurrently.
                    es_pair = []
                    emt = wk.tile([128, 2, NMT], adt, name="emt", tag="emt", bufs=2)
                    for hh in range(2):
                        es_pair.append(
                            wk.tile([128, 4 * NS], adt, name="es", tag=f"es{hh}", bufs=2)
                        )
                    for half in range(2):
                        pss = [
                            ppv.tile([128, 1024], _F32, name="ps", tag="pv")
                            for _ in range(2)
                        ]
                        if half == 0:
                            for hh in range(2):
                                b0 = hh * 64
                                nc.tensor.matmul(
                                    pss[hh][:, 836:964],
                                    kt[b0:b0 + 64, 0:NMT],
                                    qc[b0:b0 + 64, 0:NMT],
                                    start=True, stop=True,
                                    tile_position=(b0, 0),
                                    skip_group_check=True,
                                )
                        for j in range(2):
                            k0, k1 = TCH[half * 2 + j]
                            for hh in range(2):
                                b0 = hh * 64
                                nc.tensor.matmul(
                                    pss[hh][0:k1 - k0, j * 512:j * 512 + NS],
                                    kt[b0:b0 + 64, k0:k1],
                                    qc[b0:b0 + 64, NMT:N],
                                    start=True, stop=True,
                                    tile_position=(b0, 0),
                                    skip_group_check=True,
                                )
                        for hh in range(2):
                            nc.scalar.activation(
                                es_pair[hh].rearrange("p (b k) -> p b k", k=NS)[
                                    :, 2 * half:2 * half + 2, :
                                ],
                                pss[hh].rearrange("p (b k) -> p b k", k=512)[
                                    :, 0:2, 0:NS
                                ],
                                Exp,
                                scale=SCALE,
                            )
                            if half == 0:
                                nc.scalar.activation(
                                    emt[:, hh, :],
                                    pss[hh][:, 836:964],
                                    Exp,
                                    scale=SCALE,
                                )

                    # PV col-tiled: h0 -> partitions 0:64, h1 -> 64:128 of one
                    # tile; attn.V in bank0 (cols 0:452), denominators from
                    # the ones stationary in bank1 (cols 512:964).
                    pv2 = ppv.tile([128, 1024], _F32, name="pv2", tag="pv")
                    for hh in range(2):
                        h = 2 * p + hh
                        cpos = hh * 64
                        nc.tensor.matmul(
                            pv2[cpos:cpos + 64, 0:NMT],
                            vt[0][0:NMT, h * 64:(h + 1) * 64],
                            emt[:, hh, :],
                            start=True, stop=True,
                            tile_position=(0, cpos),
                            skip_group_check=True,
                        )
                    for hh in range(2):
                        cpos = hh * 64
                        nc.tensor.matmul(
                            pv2[cpos:cpos + 64, 512:512 + NMT],
                            ones[0:NMT, 0:64],
                            emt[:, hh, :],
                            start=True, stop=True,
                            tile_position=(0, cpos),
                            skip_group_check=True,
                        )
                    for kcj in range(4):
                        k0, k1 = TCH[kcj]
                        ksz = k1 - k0
                        for hh in range(2):
                            h = 2 * p + hh
                            cpos = hh * 64
                            nc.tensor.matmul(
                                pv2[cpos:cpos + 64, NMT:N],
                                vt[kcj][0:ksz, h * 64:(h + 1) * 64],
                                es_pair[hh][0:ksz, kcj * NS:(kcj + 1) * NS],
                                start=(kcj == 0), stop=(kcj == 3),
                                tile_position=(0, cpos),
                                skip_group_check=True,
                            )
                        for hh in range(2):
                            cpos = hh * 64
                            nc.tensor.matmul(
                                pv2[cpos:cpos + 64, 512 + NMT:512 + N],
                                ones[0:ksz, 0:64],
                                es_pair[hh][0:ksz, kcj * NS:(kcj + 1) * NS],
                                start=(kcj == 0), stop=(kcj == 3),
                                tile_position=(0, cpos),
                                skip_group_check=True,
                            )
                    # normalize both heads at once (128 partitions)
                    r = wk.tile([128, N], _F32, name="r", tag="r", bufs=2)
                    nc.vector.reciprocal_approx_fast(r[:, :], pv2[:, 512:512 + N])
                    at = wk.tile([128, N], pdt, name=f"attnT{p}", tag=f"attnT{p}", bufs=2)
                    nc.vector.tensor_mul(at[:, :], pv2[:, 0:N], r[:, :])
                    attnT.append(at)

                # ---- output projection (bias added on host) ----
                for (q0, q1) in TCH:
                    qsz = q1 - q0
                    pya = pacc.tile([128, 512], _F32, name="pya", tag="acc")
                    pyb = pacc.tile([128, 512], _F32, name="pyb", tag="acc")
                    for mc in range(KC):
                        nc.tensor.matmul(
                            pya[0:qsz, 0:512],
                            attnT[mc][:, q0:q1],
                            wp_sb[mc][:, 0:512],
                            start=(mc == 0), stop=(mc == KC - 1),
                        )
                        nc.tensor.matmul(
                            pyb[0:qsz, 0:256],
                            attnT[mc][:, q0:q1],
                            wp_sb[mc][:, 512:768],
                            start=(mc == 0), stop=(mc == KC - 1),
                        )
                    ysb = wk.tile([128, C], _F32, name="ysb", tag="ysb", bufs=3)
                    nc.vector.tensor_copy(ysb[0:qsz, 0:512], pya[0:qsz, 0:512])
                    nc.vector.tensor_copy(ysb[0:qsz, 512:768], pyb[0:qsz, 0:256])
                    nc.sync.dma_start(y_d[s, q0:q1, :], ysb[0:qsz, :])

    _legalize_waits(nc)
    return nc


_NC_CACHE = {}


def _get_module():
    key = (PROJ_DT_NAME, ATT_DT_NAME, PHASES, REPS)
    if key not in _NC_CACHE:
        _NC_CACHE[key] = build_module()
    return _NC_CACHE[key]


def kernel(x, temp_mask, W_qkv, W_proj, b_proj, t_h=None, t_w=None, s_h=None, s_w=None):
    x = np.asarray(x, dtype=np.float32)
    temp_mask = np.asarray(temp_mask, dtype=np.float32)
    B = x.shape[0]
    assert x.shape == (32, N, C), x.shape

    pdt_np = ml_dtypes.bfloat16 if PROJ_DT_NAME == "bfloat16" else np.float32
    wqkvT = np.ascontiguousarray(np.asarray(W_qkv, np.float32).T).astype(pdt_np)
    wpT = np.ascontiguousarray(np.asarray(W_proj, np.float32).T).astype(pdt_np)
    tm = np.ascontiguousarray(temp_mask.reshape(B, 64))

    nc = _get_module()
    per = B // NCORES
    in_maps = [
        {
            "x": np.ascontiguousarray(x[c * per:(c + 1) * per]),
            "tmask": np.ascontiguousarray(tm[c * per:(c + 1) * per]),
            "wqkvT": wqkvT,
            "wpT": wpT,
        }
        for c in range(NCORES)
    ]
    res = run_bass_kernel_spmd(nc, in_maps, core_ids=list(range(NCORES)), trace=TRACE)
    kernel.last_result = res
    y = np.concatenate([res.results[c]["y"] for c in range(NCORES)], axis=0)
    y = y + np.asarray(b_proj, np.float32)[None, None, :]
    return y.astype(np.float32)
